# revision 27
# baseline (speedup 1.0000x reference)
"""Trainium2 Bass kernel for a 4-head spatial MultiHeadAttention block.

Reference computation (per batch n):
    q/k/v = 1x1-conv projections of x (C=256 channels, S=48*48=2304 positions)
    per head (4 heads, d=64): attn = softmax(q^T k / 8), out = attn @ v
    out = Wo @ concat(heads) + bo + x   (residual)

Sharding across 8 NeuronCores: core c handles batch n = c//2 and head-pair
hp = c%2 (output channels [hp*128, hp*128+128) of the QKV projections, i.e.
heads {2*hp, 2*hp+1}).  Each core computes a partial output
Wo[:, ch] @ attn_ch (256 x 2304); the host sums the two partials per batch
and adds bo + Wo@bv + residual x.

Fp8 redesign of the ~116 us bf16 pipeline (key changes):
  - Attention processed in batches of TWO t-tiles (1024-wide PSUM groups,
    sc pool 2 banks x 3 bufs + ot 1 bank x 2 bufs = all 8 PSUM banks).
  - Every attn@V is ONE DoubleRow fp8 matmul contracting both t-tiles
    (K=256) in a single pass: PE time for attn@V halves vs bf16.
  - exp batches ALTERNATE whole-batch between engines: even batches run
    exact exp on ScalarE (ACT, out fp8e4, bias=-2 inside the exp keeps
    max value ~93 < 240 so e4m3 can't overflow), odd batches run on
    VectorE via an int8 Schraudolph trick (e5m2 bits of exp(s/8-2) =
    s*0.72135 + 47.97).  The e^-2 scale is common to every element of a
    softmax row, so normalization cancels it exactly.  Rowsums are
    accumulated from the SAME fp8 values by ones-columns in vt, keeping
    softmax consistent (sim: rel err ~3e-3 vs 2e-2 budget).
  - vt stored fp8e4 as [P, tt, 256] so the DoubleRow stationary AP's
    middle step (256 B) meets the %16 rule: [dA |1A |1B |0*63 |dB |pad].
  - bv is folded out of the device kernel: softmax rows sum to 1, so
    v-bias contributes Wo@bv, added on the host with bo.
  - Projection prologue in 1024-wide PSUM groups (fewer, wider bias adds).
All matmul operands bf16 except attn@V (fp8); accumulation fp32.
"""

import numpy as np

import concourse.bass as bass
import concourse.mybir as mybir
import concourse.tile as tile
from concourse import bacc
from concourse.bass_utils import run_bass_kernel_spmd

C = 256          # channels
S = 2304         # spatial positions (48*48)
HD = 64          # head dim
P = 128          # partitions
TT = S // P      # 18 t-tiles of 128
NG = TT // 2     # 9 two-tile batches per (chunk, head) unit
SCALE = 0.125    # 1/sqrt(HD)
F32 = mybir.dt.float32
BF16 = mybir.dt.bfloat16
F8E4 = mybir.dt.float8e4
F8E5 = mybir.dt.float8e5
I8 = mybir.dt.int8
EXP_BIAS = -2.0  # exp(u-2): e4m3-safe max ~93, cancels in normalization
# e5m2 Schraudolph on raw scores s: bits = s*(0.5/ln2) + 4*(15 - 2/ln2) - corr
EXP5_A = 0.5 / np.log(2.0)
EXP5_B = 4.0 * (15.0 + EXP_BIAS / np.log(2.0)) - 0.5

S_CHUNKS = [(0, 512), (512, 512), (1024, 512), (1536, 512), (2048, 256)]
X_PIECES = [(0, 256), (256, 256), (512, 512), (1024, 512), (1536, 768)]
# wqkv slot indices: [wk_a0, wk_a1, wq_a0, wq_a1, wv_a0, wv_a1]
WK0, WK1, WQ0, WQ1, WV0, WV1 = range(6)
DR = mybir.MatmulPerfMode.DoubleRow


def _body(tc):
    nc = tc.nc
    t_x = [nc.dram_tensor(f"x{i}", [P, 2 * pw], F8E4, kind="ExternalInput").ap()
           for i, (_, pw) in enumerate(X_PIECES)]
    t_wqkv = nc.dram_tensor("wqkv", [P, 6 * P], F8E4, kind="ExternalInput").ap()
    t_wot = nc.dram_tensor("wot", [P, C], BF16, kind="ExternalInput").ap()
    t_bq = nc.dram_tensor("bq", [P, 1], F32, kind="ExternalInput").ap()
    t_bk = nc.dram_tensor("bk", [P, 1], F32, kind="ExternalInput").ap()
    t_out = nc.dram_tensor("out", [2, P, S], BF16, kind="ExternalOutput").ap()

    singles = tc.alloc_tile_pool(name="singles", bufs=1)
    x_sb = [singles.tile([P, 2 * pw], F8E4, name=f"x_sb{i}")
            for i, (_, pw) in enumerate(X_PIECES)]
    q_sb = singles.tile([P, S], BF16)
    kz0 = singles.tile([P, S], BF16)          # head A rows 0-63, zeros 64-127
    kz1 = singles.tile([P, S], BF16)          # zeros 0-63, head B rows 64-127
    # vt cols per tt: [dA(0:64) | 1A(64) | 1B(65) | 0(66:129) | dB(129:193) | junk]
    vt_sb = singles.tile([P, TT, 256], F8E4)
    wqkv_sb = singles.tile([P, 6 * P], F8E4)
    wot_sb = singles.tile([P, C], BF16)
    attn_full = singles.tile([P, S], BF16)
    ob = singles.tile([P, 2, S], BF16)        # output staging [p, half, s]
    bq_sb = singles.tile([P, 1], F32)
    bk_sb = singles.tile([P, 1], F32)
    scr = singles.tile([1, 1], F32)
    eb_sb = singles.tile([P, 1], F32)         # exp bias constant (EXP_BIAS)
    dum_w = singles.tile([P, P], BF16)
    dum_m = singles.tile([P, 512], BF16)

    def xs(s0, sw):
        """x pair view [P, 2, sw] (both c-halves) for absolute s-range start
        s0 (range must stay inside one piece)."""
        for i, (p0, pw) in enumerate(X_PIECES):
            if p0 <= s0 < p0 + pw:
                off = s0 - p0
                x3 = x_sb[i].rearrange("p (a s) -> p a s", a=2)
                return x3[:, :, off:off + sw]
        raise AssertionError(s0)

    wq3 = wqkv_sb.rearrange("p (w d) -> p w d", d=P)

    # warm-up operands before anything else on GpSimd (no DMA dependency)
    nc.gpsimd.memset(dum_w, 0.0)
    nc.gpsimd.memset(dum_m, 0.0)
    nc.gpsimd.memset(eb_sb, EXP_BIAS)
    # ---- input DMAs: wk+wq first (gate the prologue), then x pieces in
    # need-order; late-needed weights last ----
    nc.gpsimd.dma_start(out=wqkv_sb[:, 0:2 * P], in_=t_wqkv[:, 0:2 * P])
    nc.sync.dma_start(out=x_sb[0], in_=t_x[0])
    nc.gpsimd.dma_start(out=wqkv_sb[:, 2 * P:4 * P], in_=t_wqkv[:, 2 * P:4 * P])
    nc.gpsimd.dma_start(out=bk_sb, in_=t_bk)
    nc.gpsimd.dma_start(out=bq_sb, in_=t_bq)
    for i in range(1, len(X_PIECES)):
        nc.sync.dma_start(out=x_sb[i], in_=t_x[i])
    nc.gpsimd.dma_start(out=wqkv_sb[:, 4 * P:6 * P], in_=t_wqkv[:, 4 * P:6 * P])
    # pin the exp table set now; input is a self-zeroed scratch (no DMA dep)
    nc.scalar.memzero(scr)
    nc.scalar.activation(scr, scr, mybir.ActivationFunctionType.Exp)
    nc.gpsimd.dma_start(out=wot_sb, in_=t_wot)
    # dead K halves + VT ones/zeros columns; GpSimd is idle at startup
    nc.gpsimd.memset(kz0[HD:P, :], 0.0)
    nc.gpsimd.memset(kz1[0:HD, :], 0.0)
    nc.gpsimd.memset(vt_sb[:, :, HD:HD + 2], 1.0)
    nc.gpsimd.memset(vt_sb[:, :, HD + 2:129], 0.0)

    ps = tc.alloc_tile_pool(name="ps", bufs=1, space="PSUM")
    ex_pool = tc.alloc_tile_pool(name="ex_sb", bufs=2)
    nrm = tc.alloc_tile_pool(name="nrm", bufs=2)

    def sc_tile(name):
        return ps.tile([P, 1024], F32, tag="sc", bufs=3, name=name)

    def ot_tile(name):
        return ps.tile([P, 512], F32, tag="ot", bufs=2, name=name)

    # ~10 junk matmuls while the input DMAs land: keeps the PE busy through
    # the HAM activity window so the real prologue runs at 2.4 GHz
    for _ in range(10):
        wps = sc_tile("warm")[:, :512]
        nc.tensor.matmul(wps, dum_w, dum_m, start=True, stop=True)

    def kq_multi(kind, ranges, tag="sc"):
        # K or Q projection pieces sharing one PSUM alloc (offsets must keep
        # each matmul output inside the alloc)
        w0, w1 = (WK0, WK1) if kind == "k" else (WQ0, WQ1)
        base = ranges[0][0]
        wtot = sum(r[1] for r in ranges)
        psn = (sc_tile(kind + "ps") if tag == "sc" else ot_tile(kind + "ps"))[:, :wtot]
        for s0, sw in ranges:
            pw = psn[:, s0 - base:s0 - base + sw]
            nc.tensor.matmul(pw, wq3[:, w0:w0 + 2, :], xs(s0, sw),
                             start=True, stop=True, perf_mode=DR)
        if kind == "k":
            # ScalarE is idle before the first exp; let it carry half the adds
            nc.scalar.add(kz0[0:HD, base:base + wtot], psn[0:HD, :],
                          bk_sb[0:HD, :])
            if wtot >= 1024:  # widest group on ScalarE too (DVE is tighter)
                nc.scalar.add(kz1[HD:P, base:base + wtot], psn[HD:P, :],
                              bk_sb[HD:P, :])
            else:
                nc.vector.tensor_scalar_add(kz1[HD:P, base:base + wtot],
                                            psn[HD:P, :], bk_sb[HD:P, :])
        else:
            nc.scalar.add(q_sb[:, base:base + wtot], psn, bq_sb)

    def vt_multi(base, n, tag="sc"):
        # n consecutive VT t-tiles in one PSUM alloc; fp8 copies out (bv is
        # folded into the host epilogue, so no bias add here)
        psn = sc_tile("vtps") if tag == "sc" else ot_tile("vtps")
        ps3 = psn[:, :n * P].rearrange("p (n d) -> p n d", d=P)
        for j in range(n):
            tt = base + j
            nc.tensor.matmul(ps3[:, j, :], xs(tt * P, P),
                             wq3[:, WV0:WV0 + 2, :],
                             start=True, stop=True, perf_mode=DR)
        va = vt_sb[:, base:base + n, 0:HD]
        vb = vt_sb[:, base:base + n, 129:193]
        pa = bass.AP(tensor=ps3.tensor, offset=ps3.offset,
                     ap=[ps3.ap[0], ps3.ap[1], [ps3.ap[2][0], HD]])
        pb_src = ps3[:, :, HD:P]
        nc.scalar.copy(va, pa)
        nc.vector.tensor_copy(vb, pb_src)

    def emit_av(pend):
        # one DoubleRow fp8 matmul per 2-t-tile batch (K=256 per pass).
        # head A ot rows: [dA 0:64 | rowsum 64]; head B: [rowsum 0 | 0 | dB]
        ex, g, ot, h, sw = pend[:5]
        st = vt_sb[:, 2 * g:2 * g + 2, 0:65] if h == 0 \
            else vt_sb[:, 2 * g:2 * g + 2, 65:193]
        o = ot[:, :sw] if h == 0 else ot[0:P, :sw]
        ex3 = ex.rearrange("p (two s) -> p two s", two=2)
        nc.tensor.matmul(o, st, ex3, start=(g == 0), stop=(g == NG - 1),
                         perf_mode=DR)

    def wo_chunk(ci):
        s0, sw = S_CHUNKS[ci]
        psn = sc_tile("wops")
        for half in range(2):
            pw = psn[:, half * 512:half * 512 + sw]
            cs = slice(half * P, (half + 1) * P)
            nc.tensor.matmul(pw, wot_sb[:, cs], attn_full[:, s0:s0 + sw],
                             start=True, stop=True)
        # one strided copy for both halves; alternate engines across chunks
        pv = bass.AP(tensor=psn.tensor, offset=psn.offset,
                     ap=[psn.ap[0], [512, 2], [psn.ap[1][0], sw]])
        if ci == 4:
            nc.vector.tensor_copy(ob[:, :, s0:s0 + sw], pv)
        else:
            nc.scalar.copy(ob[:, :, s0:s0 + sw], pv)

    def out_dma(s0, sw):
        # all output DMA issues ride the (mostly idle) sync queue so the
        # ScalarE pipeline never stalls on a descriptor build
        for half in range(2):
            nc.sync.dma_start(out=t_out[half, :, s0:s0 + sw],
                              in_=ob[:, half, s0:s0 + sw])

    def emit_norm(ot, h, s0, sw):
        rinv = nrm.tile([1, 512], F32, tag="rinv", name="rinv")[:, :sw]
        if h == 0:
            # head A row-sum lives on ot partition 64: ScalarE stages just
            # that row to SBUF, a 1-row DMA hops it to partition 0 (recip and
            # broadcast need base 0), and the mul reads dA from PSUM directly
            r64 = nrm.tile([P, 512], F32, tag="r64", name="r64")[HD:HD + 1, :sw]
            nc.scalar.copy(r64, ot[HD:HD + 1, :sw])
            rs0 = nrm.tile([1, 512], F32, tag="rs0", name="rs0")[:, :sw]
            nc.sync.dma_start(out=rs0, in_=r64)
            nc.vector.reciprocal_approx_fast(rinv, rs0)
            rb = nrm.tile([HD, 512], F32, tag="rb", name="rb")[:, :sw]
            nc.gpsimd.partition_broadcast(rb, rinv)
            nc.vector.tensor_mul(attn_full[0:HD, s0:s0 + sw], ot[0:HD, :sw],
                                 rb)
            return
        # head B ot = [rowsum@0 | zeros | dB@64:128]: recip directly from
        # PSUM at base 0, multiply at base 64 straight into attn_full
        nc.vector.reciprocal_approx_fast(rinv, ot[0:1, :sw])
        rb = nrm.tile([P, 512], F32, tag="rb", name="rb")[:, :sw]
        nc.gpsimd.partition_broadcast(rb, rinv)
        nc.vector.tensor_mul(attn_full[HD:P, s0:s0 + sw], ot[HD:P, :sw],
                             rb[HD:P, :])

    # ---- dense projection prologue (ordered by x-piece arrival) ----
    kq_multi("k", [(0, 256), (256, 256)])
    kq_multi("q", [(0, 256), (256, 256)], tag="ot")
    kq_multi("k", [(512, 512), (1024, 512)])
    kq_multi("q", [(512, 512), (1024, 512)])
    vt_multi(0, 8)
    kq_multi("k", [(1536, 512), (2048, 256)])
    kq_multi("q", [(1536, 512), (2048, 256)])
    vt_multi(8, 8)
    vt_multi(16, 2, tag="ot")

    # ---- attention: software-pipelined across all (s-chunk, head) units;
    # exp batches alternate ScalarE (exact, fp8e4) / VectorE (Schraudolph,
    # fp8e5) so both engines stream concurrently on different PSUM bufs ----
    pend = None       # exp batch whose attn@V is pending
    pend_norm = None  # unit awaiting normalization
    parity = 0        # global alternation: ~equal ScalarE/VectorE batches
    unit = 0
    for ci, (s0, sw) in enumerate(S_CHUNKS):
        for h in range(2):
            unit += 1
            if unit in (4, 7):  # tilt split to ~47:43 (ScalarE is faster)
                parity = 0
            kz = kz0 if h == 0 else kz1
            ot = ot_tile("ot")
            if h == 0:
                ot = ot[0:65]
            for g in range(NG):
                sc = sc_tile("sc")[:, :2 * sw]
                for j in range(2):
                    tt = 2 * g + j
                    nc.tensor.matmul(sc[:, j * sw:(j + 1) * sw],
                                     kz[:, tt * P:(tt + 1) * P],
                                     q_sb[:, s0:s0 + sw],
                                     start=True, stop=True)
                if parity == 0:
                    ex = ex_pool.tile([P, 2 * sw], F8E4, tag="ex4", name="ex4")
                    nc.scalar.activation(ex, sc,
                                         mybir.ActivationFunctionType.Exp,
                                         scale=SCALE, bias=eb_sb)
                else:
                    ex = ex_pool.tile([P, 2 * sw], F8E5, tag="ex5", name="ex5")
                    nc.vector.tensor_scalar(out=ex.bitcast(I8), in0=sc,
                                            scalar1=EXP5_A, scalar2=EXP5_B,
                                            op0=mybir.AluOpType.mult,
                                            op1=mybir.AluOpType.add)
                parity ^= 1
                if pend is not None:
                    emit_av(pend)
                    if pend[1] == NG - 1:  # last batch of its unit
                        emit_norm(*pend_norm)
                pend = (ex, g, ot, h, sw)
                if g == NG - 1:
                    pend_norm = (ot, h, s0, sw)
    # ---- Wo + output drain: wo0 overlaps the final exp, the rest follow
    # the last attn@V; the tail chunk waits only on the last norm ----
    wo_chunk(0)
    emit_av(pend)
    wo_chunk(1)
    emit_norm(*pend_norm)
    out_dma(0, 1024)
    wo_chunk(2)
    out_dma(1024, 512)
    wo_chunk(3)
    out_dma(1536, 512)
    wo_chunk(4)
    out_dma(2048, 256)

    nrm.release()
    ex_pool.release()
    ps.release()
    singles.release()


_NC_CACHE = {}


def build_nc():
    if "nc" not in _NC_CACHE:
        nc = bacc.Bacc("TRN2", target_bir_lowering=False, debug=False, num_devices=8)
        with tile.TileContext(nc) as tc:
            _body(tc)
        nc.compile()
        _NC_CACHE["nc"] = nc
    return _NC_CACHE["nc"]


def make_in_maps(x, Wq, bq, Wk, bk, Wv, bv, Wo, bo):
    import ml_dtypes
    bf16 = ml_dtypes.bfloat16
    N = x.shape[0]
    # (N, C, S) -> per batch (P, 2, S): partition p holds rows p and p+128
    fp8 = ml_dtypes.float8_e4m3
    xf = np.asarray(x, np.float32).reshape(N, C, S).reshape(N, 2, P, S)
    xf = np.ascontiguousarray(xf.transpose(0, 2, 1, 3).astype(fp8))
    in_maps = []
    for c in range(8):
        n, hp = c // 2, c % 2
        ch = slice(hp * P, (hp + 1) * P)
        wqkv = np.empty((P, 6, P), np.float32)
        for i, W in enumerate((Wk, Wq, Wv)):
            wt = np.asarray(W, np.float32)[ch].T  # (C, 128): [c_in, d_out]
            wqkv[:, 2 * i, :] = wt[0:P]
            wqkv[:, 2 * i + 1, :] = wt[P:C]
        wot = np.asarray(Wo, np.float32)[:, ch].T  # (128, 256)
        m = {
            "wqkv": np.ascontiguousarray(wqkv.astype(fp8).reshape(P, 6 * P)),
            "wot": np.ascontiguousarray(wot.astype(bf16)),
            "bq": np.ascontiguousarray(np.asarray(bq, np.float32)[ch].reshape(P, 1)),
            "bk": np.ascontiguousarray(np.asarray(bk, np.float32)[ch].reshape(P, 1)),
        }
        for i, (p0, pw) in enumerate(X_PIECES):
            m[f"x{i}"] = np.ascontiguousarray(
                xf[n][:, :, p0:p0 + pw].reshape(P, 2 * pw))
        in_maps.append(m)
    return in_maps


def run(inputs, **kwargs):
    """Run on 8 cores; returns (full output, BassKernelResults)."""
    nc = build_nc()
    in_maps = make_in_maps(**inputs)
    res = run_bass_kernel_spmd(nc, in_maps, core_ids=list(range(8)), **kwargs)
    x = np.asarray(inputs["x"], np.float32)
    bo = np.asarray(inputs["bo"], np.float32)
    # bv folded out of the kernel: softmax weights sum to 1 -> + Wo@bv
    bias = bo + np.asarray(inputs["Wo"], np.float32) @ np.asarray(
        inputs["bv"], np.float32)
    N, _, H, W = x.shape
    out = np.empty((N, C, S), np.float32)
    for n in range(N):
        p0 = np.asarray(res.results[2 * n]["out"], np.float32).reshape(C, S)
        p1 = np.asarray(res.results[2 * n + 1]["out"], np.float32).reshape(C, S)
        out[n] = x[n].reshape(C, S) + p0 + p1 + bias[:, None]
    return out.reshape(N, C, H, W), res


def kernel(**inputs):
    out, _ = run(inputs)
    return out


# revision 31
# speedup vs baseline: 1.0709x; 1.0709x over previous
"""Trainium2 Bass kernel for a 4-head spatial MultiHeadAttention block.

Reference computation (per batch n):
    q/k/v = 1x1-conv projections of x (C=256 channels, S=48*48=2304 positions)
    per head (4 heads, d=64): attn = softmax(q^T k / 8), out = attn @ v
    out = Wo @ concat(heads) + bo + x   (residual)

Sharding across 8 NeuronCores: core c handles batch n = c//2 and head-pair
hp = c%2 (output channels [hp*128, hp*128+128) of the QKV projections, i.e.
heads {2*hp, 2*hp+1}).  Each core computes a partial output
Wo[:, ch] @ attn_ch (256 x 2304); the host sums the two partials per batch
and adds bo + Wo@bv + residual x.

Fp8 redesign of the ~116 us bf16 pipeline (key changes):
  - Attention processed in batches of TWO t-tiles (1024-wide PSUM groups,
    sc pool 2 banks x 3 bufs + ot 1 bank x 2 bufs = all 8 PSUM banks).
  - Every attn@V is ONE DoubleRow fp8 matmul contracting both t-tiles
    (K=256) in a single pass: PE time for attn@V halves vs bf16.
  - exp batches ALTERNATE whole-batch between engines: even batches run
    exact exp on ScalarE (ACT, out fp8e4, bias=-2 inside the exp keeps
    max value ~93 < 240 so e4m3 can't overflow), odd batches run on
    VectorE via an int8 Schraudolph trick (e5m2 bits of exp(s/8-2) =
    s*0.72135 + 47.97).  The e^-2 scale is common to every element of a
    softmax row, so normalization cancels it exactly.  Rowsums are
    accumulated from the SAME fp8 values by ones-columns in vt, keeping
    softmax consistent (sim: rel err ~3e-3 vs 2e-2 budget).
  - vt stored fp8e4 as [P, tt, 256] so the DoubleRow stationary AP's
    middle step (256 B) meets the %16 rule: [dA |1A |1B |0*63 |dB |pad].
  - bv is folded out of the device kernel: softmax rows sum to 1, so
    v-bias contributes Wo@bv, added on the host with bo.
  - Projection prologue in 1024-wide PSUM groups (fewer, wider bias adds).
All matmul operands bf16 except attn@V (fp8); accumulation fp32.
"""

import numpy as np

import concourse.bass as bass
import concourse.mybir as mybir
import concourse.tile as tile
from concourse import bacc
from concourse.bass_utils import run_bass_kernel_spmd

C = 256          # channels
S = 2304         # spatial positions (48*48)
HD = 64          # head dim
P = 128          # partitions
TT = S // P      # 18 t-tiles of 128
NG = TT // 2     # 9 two-tile batches per (chunk, head) unit
SCALE = 0.125    # 1/sqrt(HD)
F32 = mybir.dt.float32
BF16 = mybir.dt.bfloat16
F8E4 = mybir.dt.float8e4
F8E5 = mybir.dt.float8e5
I8 = mybir.dt.int8
EXP_BIAS = -2.0  # exp(u-2): e4m3-safe max ~93, cancels in normalization
# e5m2 Schraudolph on raw scores s: bits = s*(0.5/ln2) + 4*(15 - 2/ln2) - corr
EXP5_A = 0.5 / np.log(2.0)
EXP5_B = 4.0 * (15.0 + EXP_BIAS / np.log(2.0)) - 0.5

S_CHUNKS = [(0, 512), (512, 512), (1024, 512), (1536, 512), (2048, 256)]
X_PIECES = [(0, 256), (256, 256), (512, 512), (1024, 512), (1536, 768)]
# wqkv slot indices: [wk_a0, wk_a1, wq_a0, wq_a1, wv_a0, wv_a1]
WK0, WK1, WQ0, WQ1, WV0, WV1 = range(6)
DR = mybir.MatmulPerfMode.DoubleRow


def _body(tc):
    nc = tc.nc
    t_x = [nc.dram_tensor(f"x{i}", [P, 2 * pw], F8E4, kind="ExternalInput").ap()
           for i, (_, pw) in enumerate(X_PIECES)]
    t_wqkv = nc.dram_tensor("wqkv", [P, 6 * P], F8E4, kind="ExternalInput").ap()
    t_wot = nc.dram_tensor("wot", [P, C], BF16, kind="ExternalInput").ap()
    t_bq = nc.dram_tensor("bq", [P, 1], F32, kind="ExternalInput").ap()
    t_bk = nc.dram_tensor("bk", [P, 1], F32, kind="ExternalInput").ap()
    t_out = nc.dram_tensor("out", [2, P, S], BF16, kind="ExternalOutput").ap()

    singles = tc.alloc_tile_pool(name="singles", bufs=1)
    x_sb = [singles.tile([P, 2 * pw], F8E4, name=f"x_sb{i}")
            for i, (_, pw) in enumerate(X_PIECES)]
    q_sb = singles.tile([P, S], BF16)
    kz0 = singles.tile([P, S], BF16)          # head A rows 0-63, zeros 64-127
    kz1 = singles.tile([P, S], BF16)          # zeros 0-63, head B rows 64-127
    # vt cols per tt: [dA(0:64) | 1A(64) | 1B(65) | 0(66:129) | dB(129:193) | junk]
    vt_sb = singles.tile([P, TT, 256], F8E4)
    wqkv_sb = singles.tile([P, 6 * P], F8E4)
    wot_sb = singles.tile([P, C], BF16)
    attn_full = singles.tile([P, S], BF16)
    ob = singles.tile([P, 2, S], BF16)        # output staging [p, half, s]
    bq_sb = singles.tile([P, 1], F32)
    bk_sb = singles.tile([P, 1], F32)
    scr = singles.tile([1, 1], F32)
    eb_sb = singles.tile([P, 1], F32)         # exp bias constant (EXP_BIAS)
    dum_w = singles.tile([P, P], BF16)
    dum_m = singles.tile([P, 512], BF16)

    def xs(s0, sw):
        """x pair view [P, 2, sw] (both c-halves) for absolute s-range start
        s0 (range must stay inside one piece)."""
        for i, (p0, pw) in enumerate(X_PIECES):
            if p0 <= s0 < p0 + pw:
                off = s0 - p0
                x3 = x_sb[i].rearrange("p (a s) -> p a s", a=2)
                return x3[:, :, off:off + sw]
        raise AssertionError(s0)

    wq3 = wqkv_sb.rearrange("p (w d) -> p w d", d=P)

    # warm-up operands before anything else on GpSimd (no DMA dependency)
    nc.gpsimd.memset(dum_w, 0.0)
    nc.gpsimd.memset(dum_m, 0.0)
    nc.gpsimd.memset(eb_sb, EXP_BIAS)
    # ---- input DMAs: wk+wq first (gate the prologue), then x pieces in
    # need-order; late-needed weights last ----
    nc.gpsimd.dma_start(out=wqkv_sb[:, 0:2 * P], in_=t_wqkv[:, 0:2 * P])
    nc.sync.dma_start(out=x_sb[0], in_=t_x[0])
    nc.gpsimd.dma_start(out=wqkv_sb[:, 2 * P:4 * P], in_=t_wqkv[:, 2 * P:4 * P])
    nc.gpsimd.dma_start(out=bk_sb, in_=t_bk)
    nc.gpsimd.dma_start(out=bq_sb, in_=t_bq)
    for i in range(1, len(X_PIECES)):
        nc.sync.dma_start(out=x_sb[i], in_=t_x[i])
    nc.gpsimd.dma_start(out=wqkv_sb[:, 4 * P:6 * P], in_=t_wqkv[:, 4 * P:6 * P])
    # pin the exp table set now; input is a self-zeroed scratch (no DMA dep)
    nc.scalar.memzero(scr)
    nc.scalar.activation(scr, scr, mybir.ActivationFunctionType.Exp)
    nc.gpsimd.dma_start(out=wot_sb, in_=t_wot)
    # dead K halves + VT ones/zeros columns; GpSimd is idle at startup
    nc.gpsimd.memset(kz0[HD:P, :], 0.0)
    nc.gpsimd.memset(kz1[0:HD, :], 0.0)
    nc.gpsimd.memset(vt_sb[:, :, HD:HD + 2], 1.0)
    nc.gpsimd.memset(vt_sb[:, :, HD + 2:129], 0.0)

    ps = tc.alloc_tile_pool(name="ps", bufs=1, space="PSUM")
    ex_pool = tc.alloc_tile_pool(name="ex_sb", bufs=2)
    nrm = tc.alloc_tile_pool(name="nrm", bufs=2)

    def sc_tile(name):
        return ps.tile([P, 1024], F32, tag="sc", bufs=3, name=name)

    def ot_tile(name):
        return ps.tile([P, 512], F32, tag="ot", bufs=2, name=name)

    # ~10 junk matmuls while the input DMAs land: keeps the PE busy through
    # the HAM activity window so the real prologue runs at 2.4 GHz
    for _ in range(10):
        wps = sc_tile("warm")[:, :512]
        nc.tensor.matmul(wps, dum_w, dum_m, start=True, stop=True)

    def kq_multi(kind, ranges, tag="sc"):
        # K or Q projection pieces sharing one PSUM alloc (offsets must keep
        # each matmul output inside the alloc)
        w0, w1 = (WK0, WK1) if kind == "k" else (WQ0, WQ1)
        base = ranges[0][0]
        wtot = sum(r[1] for r in ranges)
        psn = (sc_tile(kind + "ps") if tag == "sc" else ot_tile(kind + "ps"))[:, :wtot]
        for s0, sw in ranges:
            pw = psn[:, s0 - base:s0 - base + sw]
            nc.tensor.matmul(pw, wq3[:, w0:w0 + 2, :], xs(s0, sw),
                             start=True, stop=True, perf_mode=DR)
        if kind == "k":
            # ScalarE is idle before the first exp; let it carry half the adds
            nc.scalar.add(kz0[0:HD, base:base + wtot], psn[0:HD, :],
                          bk_sb[0:HD, :])
            if wtot >= 1024:  # widest group on ScalarE too (DVE is tighter)
                nc.scalar.add(kz1[HD:P, base:base + wtot], psn[HD:P, :],
                              bk_sb[HD:P, :])
            else:
                nc.vector.tensor_scalar_add(kz1[HD:P, base:base + wtot],
                                            psn[HD:P, :], bk_sb[HD:P, :])
        else:
            nc.scalar.add(q_sb[:, base:base + wtot], psn, bq_sb)

    def vt_multi(base, n, tag="sc"):
        # n consecutive VT t-tiles in one PSUM alloc; fp8 copies out (bv is
        # folded into the host epilogue, so no bias add here)
        psn = sc_tile("vtps") if tag == "sc" else ot_tile("vtps")
        ps3 = psn[:, :n * P].rearrange("p (n d) -> p n d", d=P)
        for j in range(n):
            tt = base + j
            nc.tensor.matmul(ps3[:, j, :], xs(tt * P, P),
                             wq3[:, WV0:WV0 + 2, :],
                             start=True, stop=True, perf_mode=DR)
        va = vt_sb[:, base:base + n, 0:HD]
        vb = vt_sb[:, base:base + n, 129:193]
        pa = bass.AP(tensor=ps3.tensor, offset=ps3.offset,
                     ap=[ps3.ap[0], ps3.ap[1], [ps3.ap[2][0], HD]])
        pb_src = ps3[:, :, HD:P]
        nc.scalar.copy(va, pa)
        nc.vector.tensor_copy(vb, pb_src)

    def emit_av(pend):
        # one DoubleRow fp8 matmul per 2-t-tile batch (K=256 per pass).
        # head A ot rows: [dA 0:64 | rowsum 64]; head B: [rowsum 0 | 0 | dB]
        ex, g, ot, h, sw = pend[:5]
        st = vt_sb[:, 2 * g:2 * g + 2, 0:65] if h == 0 \
            else vt_sb[:, 2 * g:2 * g + 2, 65:193]
        o = ot[:, :sw] if h == 0 else ot[0:P, :sw]
        ex3 = ex.rearrange("p (two s) -> p two s", two=2)
        nc.tensor.matmul(o, st, ex3, start=(g == 0), stop=(g == NG - 1),
                         perf_mode=DR)

    def wo_chunk(ci):
        s0, sw = S_CHUNKS[ci]
        psn = sc_tile("wops")
        for half in range(2):
            pw = psn[:, half * 512:half * 512 + sw]
            cs = slice(half * P, (half + 1) * P)
            nc.tensor.matmul(pw, wot_sb[:, cs], attn_full[:, s0:s0 + sw],
                             start=True, stop=True)
        # one strided copy for both halves; alternate engines across chunks
        pv = bass.AP(tensor=psn.tensor, offset=psn.offset,
                     ap=[psn.ap[0], [512, 2], [psn.ap[1][0], sw]])
        if ci == 4:
            nc.vector.tensor_copy(ob[:, :, s0:s0 + sw], pv)
        else:
            nc.scalar.copy(ob[:, :, s0:s0 + sw], pv)

    def out_dma(s0, sw):
        # all output DMA issues ride the (mostly idle) sync queue so the
        # ScalarE pipeline never stalls on a descriptor build
        for half in range(2):
            nc.sync.dma_start(out=t_out[half, :, s0:s0 + sw],
                              in_=ob[:, half, s0:s0 + sw])

    def norm_steps(ot, h, s0, sw):
        """Unit normalization as a list of single-batch steps.  Each step is
        emitted one batch apart so a step's cross-engine dependencies (DMA
        hop, GpSimd broadcast) are complete before it reaches the head of
        its engine's strict-FIFO queue — no head-of-line stalls in front of
        later exp batches."""
        rinv = nrm.tile([1, 512], F32, tag="rinv", name="rinv")[:, :sw]
        rb_n = P - HD if h == 0 else P
        rb = nrm.tile([P, 512], F32, tag="rb", name="rb")[0:rb_n, :sw]
        if h == 0:
            # head A row-sum lives on ot partition 64: ScalarE stages just
            # that row to SBUF, a 1-row DMA hops it to partition 0 (recip and
            # broadcast need base 0), and the mul reads dA from PSUM directly
            r64 = nrm.tile([P, 512], F32, tag="r64", name="r64")[HD:HD + 1, :sw]
            rs0 = nrm.tile([1, 512], F32, tag="rs0", name="rs0")[:, :sw]

            def s1():
                nc.scalar.copy(r64, ot[HD:HD + 1, :sw])
                nc.sync.dma_start(out=rs0, in_=r64)

            def s2():
                nc.vector.reciprocal_approx_fast(rinv, rs0)
                nc.gpsimd.partition_broadcast(rb, rinv)

            def s3():
                nc.vector.tensor_mul(attn_full[0:HD, s0:s0 + sw],
                                     ot[0:HD, :sw], rb[0:HD, :])

            return [s1, s2, s3]
        # head B ot = [rowsum@0 | zeros | dB@64:128]: recip directly from
        # PSUM at base 0, multiply at base 64 straight into attn_full

        def s1():
            nc.vector.reciprocal_approx_fast(rinv, ot[0:1, :sw])
            nc.gpsimd.partition_broadcast(rb, rinv)

        def s2():
            nc.vector.tensor_mul(attn_full[HD:P, s0:s0 + sw], ot[HD:P, :sw],
                                 rb[HD:P, :])

        return [s1, s2]

    # ---- dense projection prologue (ordered by x-piece arrival) ----
    kq_multi("k", [(0, 256), (256, 256)])
    kq_multi("q", [(0, 256), (256, 256)], tag="ot")
    kq_multi("k", [(512, 512), (1024, 512)])
    kq_multi("q", [(512, 512), (1024, 512)])
    vt_multi(0, 8)
    kq_multi("k", [(1536, 512), (2048, 256)])
    kq_multi("q", [(1536, 512), (2048, 256)])
    vt_multi(8, 8)
    vt_multi(16, 2, tag="ot")

    # ---- attention: software-pipelined across all (s-chunk, head) units;
    # exp batches alternate ScalarE (exact, fp8e4) / VectorE (Schraudolph,
    # fp8e5) so both engines stream concurrently on different PSUM bufs ----
    pend = None       # exp batch whose attn@V is pending
    pend_norm = None  # unit awaiting normalization
    norm_q = []       # staggered norm steps, one emitted per batch
    parity = 0        # global alternation: ~equal ScalarE/VectorE batches
    unit = 0
    for ci, (s0, sw) in enumerate(S_CHUNKS):
        for h in range(2):
            unit += 1
            if unit in (4, 7):  # tilt split to ~47:43 (ScalarE is faster)
                parity = 0
            kz = kz0 if h == 0 else kz1
            ot = ot_tile("ot")
            if h == 0:
                ot = ot[0:65]
            for g in range(NG):
                sc = sc_tile("sc")[:, :2 * sw]
                for j in range(2):
                    tt = 2 * g + j
                    nc.tensor.matmul(sc[:, j * sw:(j + 1) * sw],
                                     kz[:, tt * P:(tt + 1) * P],
                                     q_sb[:, s0:s0 + sw],
                                     start=True, stop=True)
                if parity == 0:
                    ex = ex_pool.tile([P, 2 * sw], F8E4, tag="ex4", name="ex4")
                    nc.scalar.activation(ex, sc,
                                         mybir.ActivationFunctionType.Exp,
                                         scale=SCALE, bias=eb_sb)
                else:
                    ex = ex_pool.tile([P, 2 * sw], F8E5, tag="ex5", name="ex5")
                    nc.vector.tensor_scalar(out=ex.bitcast(I8), in0=sc,
                                            scalar1=EXP5_A, scalar2=EXP5_B,
                                            op0=mybir.AluOpType.mult,
                                            op1=mybir.AluOpType.add)
                parity ^= 1
                if pend is not None:
                    emit_av(pend)
                    if pend[1] == NG - 1:  # last batch of its unit
                        norm_q.extend(norm_steps(*pend_norm))
                if norm_q:
                    norm_q.pop(0)()  # one staggered norm step per batch
                pend = (ex, g, ot, h, sw)
                if g == NG - 1:
                    pend_norm = (ot, h, s0, sw)
    # ---- Wo + output drain: wo0 overlaps the final exp, the rest follow
    # the last attn@V; the tail chunk waits only on the last norm ----
    for step in norm_q:  # drain any staggered steps still queued
        step()
    norm_q = []
    wo_chunk(0)
    emit_av(pend)
    wo_chunk(1)
    for step in norm_steps(*pend_norm):
        step()
    out_dma(0, 1024)
    wo_chunk(2)
    out_dma(1024, 512)
    wo_chunk(3)
    out_dma(1536, 512)
    wo_chunk(4)
    out_dma(2048, 256)

    nrm.release()
    ex_pool.release()
    ps.release()
    singles.release()


_NC_CACHE = {}


def build_nc():
    if "nc" not in _NC_CACHE:
        nc = bacc.Bacc("TRN2", target_bir_lowering=False, debug=False, num_devices=8)
        with tile.TileContext(nc) as tc:
            _body(tc)
        nc.compile()
        _NC_CACHE["nc"] = nc
    return _NC_CACHE["nc"]


def make_in_maps(x, Wq, bq, Wk, bk, Wv, bv, Wo, bo):
    import ml_dtypes
    bf16 = ml_dtypes.bfloat16
    N = x.shape[0]
    # (N, C, S) -> per batch (P, 2, S): partition p holds rows p and p+128
    fp8 = ml_dtypes.float8_e4m3
    xf = np.asarray(x, np.float32).reshape(N, C, S).reshape(N, 2, P, S)
    xf = np.ascontiguousarray(xf.transpose(0, 2, 1, 3).astype(fp8))
    in_maps = []
    for c in range(8):
        n, hp = c // 2, c % 2
        ch = slice(hp * P, (hp + 1) * P)
        wqkv = np.empty((P, 6, P), np.float32)
        for i, W in enumerate((Wk, Wq, Wv)):
            wt = np.asarray(W, np.float32)[ch].T  # (C, 128): [c_in, d_out]
            wqkv[:, 2 * i, :] = wt[0:P]
            wqkv[:, 2 * i + 1, :] = wt[P:C]
        wot = np.asarray(Wo, np.float32)[:, ch].T  # (128, 256)
        m = {
            "wqkv": np.ascontiguousarray(wqkv.astype(fp8).reshape(P, 6 * P)),
            "wot": np.ascontiguousarray(wot.astype(bf16)),
            "bq": np.ascontiguousarray(np.asarray(bq, np.float32)[ch].reshape(P, 1)),
            "bk": np.ascontiguousarray(np.asarray(bk, np.float32)[ch].reshape(P, 1)),
        }
        for i, (p0, pw) in enumerate(X_PIECES):
            m[f"x{i}"] = np.ascontiguousarray(
                xf[n][:, :, p0:p0 + pw].reshape(P, 2 * pw))
        in_maps.append(m)
    return in_maps


def run(inputs, **kwargs):
    """Run on 8 cores; returns (full output, BassKernelResults)."""
    nc = build_nc()
    in_maps = make_in_maps(**inputs)
    res = run_bass_kernel_spmd(nc, in_maps, core_ids=list(range(8)), **kwargs)
    x = np.asarray(inputs["x"], np.float32)
    bo = np.asarray(inputs["bo"], np.float32)
    # bv folded out of the kernel: softmax weights sum to 1 -> + Wo@bv
    bias = bo + np.asarray(inputs["Wo"], np.float32) @ np.asarray(
        inputs["bv"], np.float32)
    N, _, H, W = x.shape
    out = np.empty((N, C, S), np.float32)
    for n in range(N):
        p0 = np.asarray(res.results[2 * n]["out"], np.float32).reshape(C, S)
        p1 = np.asarray(res.results[2 * n + 1]["out"], np.float32).reshape(C, S)
        out[n] = x[n].reshape(C, S) + p0 + p1 + bias[:, None]
    return out.reshape(N, C, H, W), res


def kernel(**inputs):
    out, _ = run(inputs)
    return out
